# revision 40
# baseline (speedup 1.0000x reference)
"""Trainium2 Bass kernel for nn_DeepEdgeCongestionGNN (6-layer GCN + edge MLP).

Strategy (8 NeuronCores, SPMD):
  - Nodes sharded by graph (2048 graphs = 61440 nodes per core) in natural
    order. Per layer, each core's h shard is AllGather-replicated (chunked,
    overlapped) into an fp16 table of N 256B rows laid out in a tiled
    (partition-major) format so stores/loads are 1KB-descriptor DMAs.
  - Message gather via gpsimd.dma_gather: edges sorted by (6-block
    super-block, 32K-row table window, dst block); one instruction per
    (super-block, window) gathers all of its edges' source rows (int16
    window-relative indices). Groups of 128 edges are dst-block-pure;
    aggregation = one-hot [128e, 512dst] matmuls accumulated in per-block
    PSUM banks across the 15 windows. Self-loops via snorm-scaled identity
    matmuls on the residual tiles.
  - y^T = W^T @ s^T, fused BN+ReLU on ACT, transpose back + residual add on
    PE, store, chunked AllGather into the next table.
  - Layer 5 keeps h6 feature-major (h6T); the final edge MLP then needs NO
    gathers at all: branch endpoints are strided SBUF columns of per-128-graph
    chunks; outputs assembled per-branch into contiguous stores.
"""
import sys
import types

import numpy as np

sys.path.insert(0, "/opt/trn_rl_repo")

# --- shim antenv.axon_hooks (absent in this image) so trace=True works ---
import antenv
if "antenv.axon_hooks" not in sys.modules:
    _hookmod = types.ModuleType("antenv.axon_hooks")
    _hookmod._hook = None
    def _set(h): _hookmod._hook = h
    def _get(): return _hookmod._hook
    _hookmod.set_axon_ntff_profile_hook = _set
    _hookmod.get_axon_ntff_profile_hook = _get
    sys.modules["antenv.axon_hooks"] = _hookmod
    antenv.axon_hooks = _hookmod
    try:
        from trn_agent_boot.trn_boot import _ntff_profile_via_ctypes
        _hookmod._hook = _ntff_profile_via_ctypes("/opt/axon/libaxon_pjrt.so")
    except Exception:
        pass

import concourse.bass as bass
import concourse.bacc as bacc
import concourse.mybir as mybir
import concourse.tile as tile
from concourse import library_config
from concourse.bass_utils import run_bass_kernel_spmd

F16 = mybir.dt.float16
F32 = mybir.dt.float32
I32 = mybir.dt.int32
I16 = mybir.dt.int16

NCORES = 8
NPG = 30                    # nodes per graph
G = 16384                   # graphs
N = G * NPG                 # 491520 nodes
GPC = G // NCORES           # graphs per core
NSH = GPC * NPG             # 61440 nodes per core
NBLK = NSH // 512           # 120 dst blocks of 512 nodes
LAYERS = 6
HID = 128
SPLIT = 4                   # AllGather chunks per layer
BPC = NBLK // SPLIT         # blocks per AG chunk (30)
CHSH = NSH // SPLIT         # rows per core per AG chunk (15360)
WIN = 32768                 # dma_gather window (int16-addressable rows)
NWIN = N // WIN             # 15
SB = 6                      # blocks per psum super-block
NSB = NBLK // SB            # 20
G_CAP = 8                   # max one-hot groups per (SB, window) instr
NOUT = GPC * 41             # 83968 output rows per core
NTIL = NOUT // 128          # 656
BN_EPS = 1e-5

BRANCH_U = np.array([0,0,1,2,1,1,3,5,5,6,6,6,6,8,8,9,11,11,11,11,13,15,14,17,
                     18,9,9,21,14,21,22,23,24,24,27,26,26,28,26,7,5],
                    dtype=np.int64)
BRANCH_V = np.array([1,2,3,3,4,5,5,6,7,7,8,9,27,9,10,10,12,13,15,16,14,16,17,
                     18,19,19,20,20,22,21,23,23,24,26,26,29,28,29,27,27,8],
                    dtype=np.int64)

_CACHE = {}


def _table_row(n):
    """256B-row index of global node id n in the tiled AG table layout."""
    k = n // NSH
    l = n % NSH
    b_l = l // 512
    c = b_l // BPC
    lb = b_l % BPC
    w = l % 512
    j = w // 128
    p = w % 128
    return c * (NCORES * CHSH) + k * CHSH + p * (BPC * 4) + lb * 4 + j


def _prep(x, edge_index, enc_W, enc_b, conv_W, conv_b, bn_gamma, bn_beta,
          bn_mean, bn_var, mlp_W1, mlp_b1, mlp_W2, mlp_b2):
    src = np.ascontiguousarray(edge_index[0]).astype(np.int64)
    dst = np.ascontiguousarray(edge_index[1]).astype(np.int64)

    indeg = np.bincount(dst, minlength=N).astype(np.int64)
    deg = (indeg + 1).astype(np.float32)
    dinv = (1.0 / np.sqrt(deg)).astype(np.float32)

    erow = _table_row(src)                       # table row of each edge src
    ewin = erow // WIN
    erel = (erow % WIN).astype(np.int64)
    ek = dst // NSH
    eB = (dst % NSH) // 512
    ecol = (dst % 512).astype(np.float64)
    enorm_full = dinv[src] * dinv[dst]

    # per (core, window, block) edge lists
    key = ((ek * NWIN + ewin) * NBLK + eB)
    order = np.argsort(key, kind="stable")
    key_s = key[order]
    cnts = np.bincount(key_s, minlength=NCORES * NWIN * NBLK).reshape(
        NCORES, NWIN, NBLK)
    gmax = np.maximum(1, -(-cnts.max(axis=0) // 128))     # [NWIN, NBLK]

    # schedule: for sb in NSB: for w in NWIN: groups for blocks of sb
    sched = []
    gg = 0
    cb = 0
    first_seen = set()
    last_of_block = {}
    for sb in range(NSB):
        for w in range(NWIN):
            groups = []
            for bl in range(sb * SB, (sb + 1) * SB):
                for gi in range(int(gmax[w, bl])):
                    st = bl not in first_seen
                    if st:
                        first_seen.add(bl)
                    groups.append((bl, gg, st))
                    last_of_block[bl] = gg
                    gg += 1
            assert len(groups) <= G_CAP, (sb, w, len(groups))
            sched.append(dict(sb=sb, w=w, cb=cb, groups=groups))
            cb += len(groups) * 8
    TOTG = gg
    TOTC = cb

    # per-core gather tensors
    eidx = np.zeros((NCORES, 128, TOTC), np.int16)
    edst = np.zeros((NCORES, 128, TOTG), np.float16)
    enorm = np.zeros((NCORES, 128, TOTG), np.float32)

    starts = np.zeros(NCORES * NWIN * NBLK + 1, np.int64)
    starts[1:] = np.cumsum(cnts.reshape(-1))
    # group base per (w, bl) in the shared schedule
    gbase = np.zeros((NWIN, NBLK), np.int64)
    for ins in sched:
        for bl, g_id, _ in ins["groups"]:
            if gbase[ins["w"], bl] == 0 or True:
                pass
    # recompute gbase directly: first group id of (w, bl)
    gbase[:] = -1
    for ins in sched:
        for bl, g_id, _ in ins["groups"]:
            if gbase[ins["w"], bl] < 0:
                gbase[ins["w"], bl] = g_id
    colbase = {}
    for ins in sched:
        colbase[(ins["sb"], ins["w"])] = (ins["cb"], ins["groups"][0][1])

    for k in range(NCORES):
        for w in range(NWIN):
            for bl in range(NBLK):
                c0 = starts[(k * NWIN + w) * NBLK + bl]
                c1 = starts[(k * NWIN + w) * NBLK + bl + 1]
                eids = order[c0:c1]
                cnt = c1 - c0
                g0 = gbase[w, bl]
                sb = bl // SB
                icb, ig0 = colbase[(sb, w)]
                for j in range(cnt):
                    g_id = g0 + j // 128
                    lane = j % 128
                    edst[k, lane, g_id] = ecol[eids[j]]
                    enorm[k, lane, g_id] = enorm_full[eids[j]]
                    # wrapped idx position within the instruction
                    iloc = (g_id - ig0) * 128 + lane
                    col = icb + iloc // 16
                    row = iloc % 16
                    v = erel[eids[j]]
                    for rep in range(8):
                        eidx[k, rep * 16 + row, col] = v

    # self-loop scale dinv^2, natural order, [128, NBLK*4]
    snorm = np.empty((NCORES, 128, NBLK * 4), np.float32)
    for k in range(NCORES):
        sn = (dinv * dinv)[k * NSH:(k + 1) * NSH]
        snorm[k] = sn.reshape(NBLK * 4, 128).T

    # x, natural order, transposed per core: [8, NSH]
    xT = np.empty((NCORES, 8, NSH), np.float32)
    for k in range(NCORES):
        xT[k] = x[k * NSH:(k + 1) * NSH].T

    bnscale = (bn_gamma / np.sqrt(bn_var + BN_EPS)).astype(np.float32)
    bnshift = ((conv_b - bn_mean) * bnscale + bn_beta).astype(np.float32)

    consts = dict(
        encW=enc_W.astype(np.float32),                       # [8,128]
        encb=enc_b.reshape(128, 1).astype(np.float32),
        convW=np.concatenate([conv_W[i] for i in range(LAYERS)], axis=1
                             ).astype(np.float16),           # [128, 768]
        bnscale=bnscale.T.copy(),                            # [128, 6]
        bnshift=bnshift.T.copy(),
        w1u=mlp_W1[:128].astype(np.float16),
        w1v=mlp_W1[128:].astype(np.float16),
        w2=mlp_W2.astype(np.float16),                        # [128,1]
        b1=mlp_b1.reshape(128, 1).astype(np.float32),
    )
    b2 = float(np.asarray(mlp_b2).reshape(-1)[0])
    meta = dict(sched=sched, TOTG=TOTG, TOTC=TOTC,
                last_of_block=last_of_block)
    return eidx, edst, enorm, snorm, xT, consts, b2, meta


def _build(b2, meta):
    sched = meta["sched"]
    TOTG = meta["TOTG"]
    TOTC = meta["TOTC"]

    nc = bacc.Bacc("TRN2", target_bir_lowering=False, debug=False,
                   num_devices=NCORES)

    xT_d = nc.dram_tensor("xT", [8, NSH], F32, kind="ExternalInput")
    eidx_d = nc.dram_tensor("eidx", [128, TOTC], I16, kind="ExternalInput")
    edst_d = nc.dram_tensor("edst", [128, TOTG], F16, kind="ExternalInput")
    enorm_d = nc.dram_tensor("enorm", [128, TOTG], F32, kind="ExternalInput")
    snorm_d = nc.dram_tensor("snorm", [128, NBLK * 4], F32,
                             kind="ExternalInput")
    encW_d = nc.dram_tensor("encW", [8, 128], F32, kind="ExternalInput")
    encb_d = nc.dram_tensor("encb", [128, 1], F32, kind="ExternalInput")
    convW_d = nc.dram_tensor("convW", [128, LAYERS * 128], F16,
                             kind="ExternalInput")
    bnscale_d = nc.dram_tensor("bnscale", [128, LAYERS], F32,
                               kind="ExternalInput")
    bnshift_d = nc.dram_tensor("bnshift", [128, LAYERS], F32,
                               kind="ExternalInput")
    w1u_d = nc.dram_tensor("w1u", [128, 128], F16, kind="ExternalInput")
    w1v_d = nc.dram_tensor("w1v", [128, 128], F16, kind="ExternalInput")
    w2_d = nc.dram_tensor("w2", [128, 1], F16, kind="ExternalInput")
    b1_d = nc.dram_tensor("b1", [128, 1], F32, kind="ExternalInput")

    out_d = nc.dram_tensor("outd", [NTIL, 128], F32, kind="ExternalOutput")

    hloc = [[nc.dram_tensor(f"hloc{j}_{c}", [128, BPC * 512], F16,
                            kind="Internal") for c in range(SPLIT)]
            for j in range(2)]
    h6T_d = nc.dram_tensor("h6T", [128, NSH], F16, kind="Internal")
    tab = [nc.dram_tensor(f"tab{j}", [N, 128], F16, kind="Internal",
                          addr_space="Shared") for j in range(2)]
    RG = [list(range(NCORES))]

    def st_slice(hl, b):
        # contiguous [128, 4, 128] view of block b's store slot
        t = hl[b // BPC]
        lb = b % BPC
        return t[:, lb * 512:(lb + 1) * 512].rearrange(
            "p (j f) -> p j f", j=4)

    # schedule lookup: instructions grouped per sb
    instr_by_sb = [[] for _ in range(NSB)]
    for ins in sched:
        instr_by_sb[ins["sb"]].append(ins)
    sb_cols = []
    for sb in range(NSB):
        il = instr_by_sb[sb]
        c0 = il[0]["cb"]
        c1 = il[-1]["cb"] + len(il[-1]["groups"]) * 8
        sb_cols.append((c0, c1))
    SBW_CAP = max(c1 - c0 for c0, c1 in sb_cols)

    with tile.TileContext(nc) as tc:
        with (
            tc.tile_pool(name="const", bufs=1) as cpool,
            tc.tile_pool(name="idxp", bufs=3) as ixp,
            tc.tile_pool(name="gat", bufs=10) as gp,
            tc.tile_pool(name="oh", bufs=5) as ohp,
            tc.tile_pool(name="mid", bufs=4) as mp,
            tc.tile_pool(name="fin", bufs=2) as fp,
            tc.tile_pool(name="pps", bufs=1, space="PSUM") as pps,
            tc.tile_pool(name="ppy", bufs=1, space="PSUM") as ppy,
            tc.tile_pool(name="ppt", bufs=1, space="PSUM") as ppt,
        ):
            # ---- constants ----
            iota_i = cpool.tile([128, 128], I16, tag="iotai")
            nc.gpsimd.iota(iota_i[:], pattern=[[1, 128]], base=0,
                           channel_multiplier=0)
            iota5_i = cpool.tile([128, 512], I16, tag="iota5i")
            nc.gpsimd.iota(iota5_i[:], pattern=[[1, 512]], base=0,
                           channel_multiplier=0)
            prow_i = cpool.tile([128, 128], I16, tag="prowi")
            nc.gpsimd.iota(prow_i[:], pattern=[[0, 128]], base=0,
                           channel_multiplier=1)
            # all gpsimd standard-lib ops done; switch to mlp for dma_gather
            nc.gpsimd.load_library(library_config.mlp)

            iota512 = cpool.tile([128, 512], F16, tag="iota512")
            nc.vector.tensor_copy(out=iota512[:], in_=iota5_i[:])
            ident32 = cpool.tile([128, 128], F32, tag="id32")
            nc.vector.tensor_tensor(out=ident32[:], in0=prow_i[:],
                                    in1=iota_i[:], op=mybir.AluOpType.is_equal)
            ident16 = cpool.tile([128, 128], F16, tag="id16")
            nc.vector.tensor_copy(out=ident16[:], in_=ident32[:])

            edst_sb = cpool.tile([128, TOTG], F16, tag="edst")
            nc.sync.dma_start(out=edst_sb[:], in_=edst_d[:, :])
            enorm_sb = cpool.tile([128, TOTG], F32, tag="enorm")
            nc.sync.dma_start(out=enorm_sb[:], in_=enorm_d[:, :])
            snorm_sb = cpool.tile([128, NBLK * 4], F32, tag="snorm")
            nc.sync.dma_start(out=snorm_sb[:], in_=snorm_d[:, :])
            encW_sb = cpool.tile([8, 128], F32, tag="encW")
            nc.sync.dma_start(out=encW_sb[:], in_=encW_d[:, :])
            encb_sb = cpool.tile([128, 1], F32, tag="encb")
            nc.sync.dma_start(out=encb_sb[:], in_=encb_d[:, :])
            convW_sb = cpool.tile([128, LAYERS * 128], F16, tag="convW")
            nc.sync.dma_start(out=convW_sb[:], in_=convW_d[:, :])
            bnscale_sb = cpool.tile([128, LAYERS], F32, tag="bns")
            nc.sync.dma_start(out=bnscale_sb[:], in_=bnscale_d[:, :])
            bnshift_sb = cpool.tile([128, LAYERS], F32, tag="bnsh")
            nc.sync.dma_start(out=bnshift_sb[:], in_=bnshift_d[:, :])
            w1u_sb = cpool.tile([128, 128], F16, tag="w1u")
            nc.sync.dma_start(out=w1u_sb[:], in_=w1u_d[:, :])
            w1v_sb = cpool.tile([128, 128], F16, tag="w1v")
            nc.sync.dma_start(out=w1v_sb[:], in_=w1v_d[:, :])
            w2_sb = cpool.tile([128, 1], F16, tag="w2")
            nc.sync.dma_start(out=w2_sb[:], in_=w2_d[:, :])
            b1_sb = cpool.tile([128, 1], F32, tag="b1")
            nc.sync.dma_start(out=b1_sb[:], in_=b1_d[:, :])

            def transpose_store(y_sb, rsd, dst_ap, with_res):
                psum_t = ppt.tile([128, 4, 128], F32, tag="pt")
                for j in range(4):
                    nc.tensor.matmul(
                        out=psum_t[:, j, :],
                        lhsT=y_sb[:, j * 128:(j + 1) * 128],
                        rhs=ident32[:], is_transpose=True,
                        start=True, stop=(not with_res),
                        skip_group_check=True)
                    if with_res:
                        nc.tensor.matmul(
                            out=psum_t[:, j, :], lhsT=ident16[:],
                            rhs=rsd[:, j, :], start=False, stop=True,
                            skip_group_check=True)
                t16 = mp.tile([128, 4, 128], F16, tag="t16")
                nc.vector.tensor_copy(out=t16[:], in_=psum_t[:])
                nc.sync.dma_start(out=dst_ap, in_=t16[:])

            def ag_chunk(li, c):
                src = hloc[li % 2][c]
                dstt = tab[(li + 1) % 2]
                nc.gpsimd.collective_compute(
                    "AllGather", mybir.AluOpType.bypass,
                    replica_groups=RG,
                    ins=[src[:, :]],
                    outs=[dstt[c * CHSH * NCORES:(c + 1) * CHSH * NCORES, :]],
                )

            # ---- encoder: h0 = x @ encW + encb -> hloc[1] ----
            with nc.named_scope("encoder"):
                for b in range(NBLK):
                    xt = mp.tile([8, 512], F32, tag="xt")
                    nc.sync.dma_start(out=xt[:],
                                      in_=xT_d[:, b * 512:(b + 1) * 512])
                    psum_y = ppy.tile([128, 512], F32, tag="py")
                    nc.tensor.matmul(out=psum_y[:], lhsT=encW_sb[:],
                                     rhs=xt[:], start=True, stop=True)
                    y_sb = mp.tile([128, 512], F32, tag="y_sb")
                    nc.vector.tensor_scalar_add(out=y_sb[:], in0=psum_y[:],
                                                scalar1=encb_sb[:, :])
                    transpose_store(y_sb, None, st_slice(hloc[1], b), False)
                    if (b + 1) % BPC == 0:
                        ag_chunk(-1, b // BPC)

            # final-chunk emitter (interleaved into layer 5's drain)
            n_final_emitted = [0]

            def emit_final(gb):
                ch = fp.tile([128, 3840], F16, tag="ch")
                nc.sync.dma_start(
                    out=ch[:], in_=h6T_d[:, gb * 3840:(gb + 1) * 3840])
                chv = ch[:].rearrange("p (g s) -> p s g", s=30)
                o_sb = fp.tile([1, 41 * 128], F32, tag="o_sb")
                ov = o_sb[:].rearrange("p (g e) -> p e g", e=41)
                for e in range(41):
                    psum_hf = ppy.tile([128, 512], F32, tag="py")
                    psum_h = psum_hf[:, 0:128]
                    nc.tensor.matmul(
                        out=psum_h, lhsT=w1u_sb[:],
                        rhs=chv[:, int(BRANCH_U[e]), :],
                        start=True, stop=False)
                    nc.tensor.matmul(
                        out=psum_h, lhsT=w1v_sb[:],
                        rhs=chv[:, int(BRANCH_V[e]), :],
                        start=False, stop=True)
                    hidT = mp.tile([128, 128], F16, tag="hidT")
                    nc.scalar.activation(
                        out=hidT[:], in_=psum_h,
                        func=mybir.ActivationFunctionType.Relu,
                        bias=b1_sb[:, :], scale=1.0)
                    psum_of = ppt.tile([128, 4, 128], F32, tag="pt")
                    psum_o = psum_of[0:1, 0, :]
                    nc.tensor.matmul(out=psum_o, lhsT=w2_sb[:],
                                     rhs=hidT[:], start=True, stop=True)
                    nc.vector.tensor_scalar_add(
                        out=ov[:, e, :], in0=psum_o, scalar1=b2)
                nc.sync.dma_start(
                    out=out_d[gb * 41:(gb + 1) * 41, :].rearrange(
                        "t p -> (t p)")[None, :],
                    in_=o_sb[:])

            # ---- 6 GCN layers ----
            for li in range(LAYERS):
                t_cur = tab[li % 2]
                h_prev = hloc[(li + 1) % 2]
                h_next = hloc[li % 2]
                last = (li == LAYERS - 1)
                pending_ag = []
                with nc.named_scope(f"layer{li}"):
                    for sb in range(NSB):
                        pst = pps.tile([128, SB, 512], F32, tag="pst")
                        rsds = []
                        for q in range(SB):
                            b = sb * SB + q
                            rsd = mp.tile([128, 4, 128], F16,
                                          tag=f"rsd{q % 3}")
                            nc.sync.dma_start(out=rsd[:],
                                              in_=st_slice(h_prev, b))
                            rsds.append(rsd)
                        sc0, sc1 = sb_cols[sb]
                        eix = ixp.tile([128, SBW_CAP], I16, tag="eix")
                        nc.sync.dma_start(out=eix[:, 0:sc1 - sc0],
                                          in_=eidx_d[:, sc0:sc1])
                        for ins in instr_by_sb[sb]:
                            if pending_ag and ins["w"] >= 4:
                                # trigger behind this sb's first gathers so
                                # the store-completion wait doesn't stall them
                                ag_chunk(li, pending_ag.pop(0))
                            w = ins["w"]
                            ng = len(ins["groups"])
                            off = ins["cb"] - sc0
                            ig0 = ins["groups"][0][1]
                            Msl = gp.tile([128, G_CAP, 128], F16, tag="M")
                            nc.gpsimd.dma_gather(
                                Msl[:, 0:ng, :],
                                t_cur[w * WIN:(w + 1) * WIN, :],
                                eix[:, off:off + ng * 8], ng * 128,
                                ng * 128, 128)
                            nb = enorm_sb[:, ig0:ig0 + ng][:, :, None] \
                                .to_broadcast([128, ng, 128])
                            nc.vector.tensor_tensor(
                                out=Msl[:, 0:ng, :], in0=Msl[:, 0:ng, :],
                                in1=nb, op=mybir.AluOpType.mult)
                            Ssl = ohp.tile([128, G_CAP, 512], F16, tag="S")
                            db = edst_sb[:, ig0:ig0 + ng][:, :, None] \
                                .to_broadcast([128, ng, 512])
                            ib = iota512[:][:, None, :] \
                                .to_broadcast([128, ng, 512])
                            nc.vector.tensor_tensor(
                                out=Ssl[:, 0:ng, :], in0=db, in1=ib,
                                op=mybir.AluOpType.is_equal)
                            for gi, (bl, g_id, st) in enumerate(ins["groups"]):
                                nc.tensor.matmul(
                                    out=pst[:, bl - sb * SB, :],
                                    lhsT=Msl[:, gi, :], rhs=Ssl[:, gi, :],
                                    start=st, stop=False,
                                    skip_group_check=True)
                        # residual (self-loop) matmuls + drain per block
                        for q in range(SB):
                            b = sb * SB + q
                            rsd = rsds[q]
                            rsdsc = mp.tile([128, 4, 128], F16, tag="rsdsc")
                            for j in range(4):
                                nc.scalar.activation(
                                    out=rsdsc[:, j, :], in_=rsd[:, j, :],
                                    func=mybir.ActivationFunctionType.Copy,
                                    bias=0.0,
                                    scale=snorm_sb[:, b * 4 + j:
                                                   b * 4 + j + 1])
                                nc.tensor.matmul(
                                    out=pst[:, q, j * 128:(j + 1) * 128],
                                    lhsT=rsdsc[:, j, :], rhs=ident16[:],
                                    start=False, stop=True,
                                    skip_group_check=True)
                            s_sb = mp.tile([128, 512], F16, tag="s_sb")
                            nc.vector.tensor_copy(out=s_sb[:],
                                                  in_=pst[:, q, :])
                            psum_y = ppy.tile([128, 512], F32, tag="py")
                            nc.tensor.matmul(
                                out=psum_y[:],
                                lhsT=convW_sb[:, li * 128:(li + 1) * 128],
                                rhs=s_sb[:], start=True, stop=True)
                            y_sb = mp.tile([128, 512], F32, tag="y_sb")
                            nc.scalar.activation(
                                out=y_sb[:], in_=psum_y[:],
                                func=mybir.ActivationFunctionType.Relu,
                                bias=bnshift_sb[:, li:li + 1],
                                scale=bnscale_sb[:, li:li + 1])
                            if not last:
                                transpose_store(y_sb, rsd,
                                                st_slice(h_next, b), True)
                                if (b + 1) % BPC == 0:
                                    pending_ag.append(b // BPC)
                            else:
                                # h6T = y + rsd^T, feature-major
                                rsd32 = mp.tile([128, 4, 128], F32,
                                                tag="rsd32")
                                nc.vector.tensor_copy(out=rsd32[:],
                                                      in_=rsd[:])
                                psum_t = ppt.tile([128, 4, 128], F32,
                                                  tag="pt")
                                for j in range(4):
                                    nc.tensor.matmul(
                                        out=psum_t[:, j, :],
                                        lhsT=rsd32[:, j, :], rhs=ident32[:],
                                        is_transpose=True,
                                        start=True, stop=True,
                                        skip_group_check=True)
                                h6c = mp.tile([128, 512], F16, tag="h6c")
                                nc.vector.tensor_tensor(
                                    out=h6c[:], in0=y_sb[:],
                                    in1=psum_t[:].rearrange(
                                        "p j f -> p (j f)"),
                                    op=mybir.AluOpType.add)
                                nc.sync.dma_start(
                                    out=h6T_d[:, b * 512:(b + 1) * 512],
                                    in_=h6c[:])
                        if last:
                            done = (sb + 1) * SB
                            while (n_final_emitted[0] < 16 and
                                   ((n_final_emitted[0] + 1) * 15 + 1) // 2
                                   <= done):
                                emit_final(n_final_emitted[0])
                                n_final_emitted[0] += 1
                    for c in pending_ag:
                        ag_chunk(li, c)

            # ---- final edge MLP: strided reads of h6T, no gathers ----
            for gb in range(n_final_emitted[0], 16):
                emit_final(gb)

    nc.finalize()
    return nc


def kernel(**inputs):
    x = np.asarray(inputs["x"], np.float32)
    edge_index = np.asarray(inputs["edge_index"])
    assert x.shape == (N, 8)
    eidx, edst, enorm, snorm, xT, consts, b2, meta = _prep(
        x, edge_index,
        np.asarray(inputs["enc_W"], np.float32),
        np.asarray(inputs["enc_b"], np.float32),
        np.asarray(inputs["conv_W"], np.float32),
        np.asarray(inputs["conv_b"], np.float32),
        np.asarray(inputs["bn_gamma"], np.float32),
        np.asarray(inputs["bn_beta"], np.float32),
        np.asarray(inputs["bn_mean"], np.float32),
        np.asarray(inputs["bn_var"], np.float32),
        np.asarray(inputs["mlp_W1"], np.float32),
        np.asarray(inputs["mlp_b1"], np.float32),
        np.asarray(inputs["mlp_W2"], np.float32),
        np.asarray(inputs["mlp_b2"], np.float32))

    key = "nc"
    if key not in _CACHE:
        _CACHE[key] = _build(b2, meta)
    nc = _CACHE[key]

    in_maps = []
    for k in range(NCORES):
        in_maps.append({
            "xT": xT[k], "eidx": eidx[k], "edst": edst[k],
            "enorm": enorm[k], "snorm": snorm[k],
            "encW": consts["encW"], "encb": consts["encb"],
            "convW": consts["convW"], "bnscale": consts["bnscale"],
            "bnshift": consts["bnshift"], "w1u": consts["w1u"],
            "w1v": consts["w1v"], "w2": consts["w2"], "b1": consts["b1"],
        })

    trace = bool(int(__import__("os").environ.get("KERNEL_TRACE", "0")))
    res = run_bass_kernel_spmd(nc, in_maps, core_ids=list(range(NCORES)),
                               trace=trace)
    kernel.last_result = res
    out = np.concatenate([res.results[k]["outd"].reshape(NOUT, 1)
                          for k in range(NCORES)], axis=0)
    return out.astype(np.float32)
